# revision 13
# baseline (speedup 1.0000x reference)
"""Cross-attention Trainium2 kernel: build, host prep/gather, emulation.

Sharding: 8 cores = 4 batches x 2 head-halves. Core c=(b,j) computes
heads j*8..j*8+8 for batch b, producing a partial out.T [C, N]; host
sums the two partials per batch and adds bias.

All matmuls run in fp16 (PSUM accumulation f32). Contraction dims sit
on SBUF partitions via host-side transposes:
  kT[o,m] = wkT.T @ cT ; v[m,o] = cT.T @ wvT ; qT[o,n] = wqT.T @ xT
  RoPE: pair-partner lives 16 partitions away inside each 32-partition
    quadrant (host permutes W columns accordingly) so one DVE
    stream_shuffle (on a uint32 view) fetches it; q' = q*cos + shuf(q)*sin.
    RoPE runs INSIDE phase 1, per 512-block, right after each eviction,
    so the DVE is free for exp during attention.
  Phase 1 order: K+V proj (fused, shared context loads) then Q proj
    (2 PSUM banks, two c4-pair passes) so attention can overlap the Q
    tail. Attention SBUF pools open BEFORE phase-1 pools (disjoint
    regions -> no false deps); PSUM map: ps1 psk 0-1/psv 2-5, then
    psP 0-1 (Q's banks, freed last), psO 2-3, psS 4-7 (free at KV end).
  Attention: one flat software pipeline over (nh, c4) iterations.
    S.T tile [m,n] = kT_h.T @ qT_h  (K=64, NBL=512 wide); the two heads'
    S matmuls land in ONE two-bank PSUM tile psS[:, h, :] and run
    concurrently on disjoint 64-row PE groups (tile_position auto).
    S pairs are emitted in 2-mc batches so the 64-row<->128-row tiling
    mode switch (which drains the PE array) happens twice per 2 mc.
    The next iteration's first two S/exp groups are emitted at the tail
    of the current iteration, so the PE never waits on the exp warmup.
    exp: ONE op per mc covering both heads [128, 1024] (the psS tile
    spans 2 adjacent PSUM banks), alternating engines by mc: ACT Exp
    activation vs DVE Schraudolph bit-trick writing bf16 bits through
    an int16 cast: bf16_bits(e^x) ~= int16(184.665*x + 16250.75).
  eS/vAll are bf16 (exp spans e^-13..e^13 -- fp16 would overflow);
    the q/k/S path and attnT/wp are fp16 for mantissa precision.
  PV stationary per head is 128 wide: col 0 = ones (softmax denominator
    lands on psO partition 0, where reciprocal_approx_fast can read it
    directly), cols 64..127 = v (attn rows at 64-aligned partition base;
    engine APs require 32-aligned bases). psO[p,n] += [1|..|v_h].T @ expS.
  attnT = psO[64:128] * bcast(recip(psO[0])) on DVE/gpsimd.
  outT[e, n] = wpT.T @ attnT, emitted per nh right after attention(nh)
    so out-proj matmuls/DMAs overlap attention(nh+1).
"""

import sys

sys.path.insert(0, "/opt/trn_rl_repo")

import numpy as np
import ml_dtypes

import concourse.bass as bass
import concourse.tile as tile
from concourse import bacc, mybir
from concourse.bass_utils import run_bass_kernel_spmd

P = 128
SHUFFLE_MASK = [(i + 16) % 32 for i in range(32)]
F32 = mybir.dt.float32
F16 = mybir.dt.float16
BF16 = mybir.dt.bfloat16
I16 = mybir.dt.int16
U32 = mybir.dt.uint32
F16NP = np.float16
BFNP = ml_dtypes.bfloat16

# bf16 Schraudolph: bf16_bits(e^x) ~= int16(EXP_A*x + EXP_B)
# (bf16 target: exponent covers |x| up to ~30 -- scores reach |x|~13)
EXP_A = 184.6649652337873
EXP_B = 16250.75
# mc chunks whose exp (both heads) runs on DVE via Schraudolph; the rest
# on ACT. 7/16 on DVE: DVE also carries the normalize ops.
DVE_MCS = frozenset({1, 3, 5, 7, 9, 11, 13})


class CFG:
    def __init__(self, N=2048, M=2048):
        self.N, self.M = N, M
        self.C = 1024
        self.H = 16
        self.D = 64
        self.O = 512           # local head dim total (8 heads x 64)
        self.CC = self.C // P  # 8 c-chunks
        self.OC4 = self.O // P  # 4 o-chunks
        self.HPC = 8           # heads per core
        self.NPB = min(512, N)   # proj n-block
        self.NBL = min(512, N)   # attention n-block
        self.scale = self.D ** -0.5


def perm64():
    """Device partition row p (within a head's 64) -> original component."""
    out = []
    for p in range(64):
        q2, i = divmod(p, 32)
        pair = q2 * 16 + (i % 16)
        out.append(2 * pair + (0 if i < 16 else 1))
    return np.array(out)


def rope_tables(fc, L):
    """cos/sin tables [128, L] matching the permuted q/k layout."""
    cos = np.empty((P, L), np.float32)
    sin = np.empty((P, L), np.float32)
    for p in range(P):
        p64 = p % 64
        pair = (p64 // 32) * 16 + (p64 % 16)
        is_even = (p64 % 32) < 16
        cos[p] = fc[:L, pair, 0]
        sin[p] = fc[:L, pair, 1] * (-1.0 if is_even else 1.0)
    return cos, sin


def host_prep(x, context, freqs_cis, Wq, Wkv, Wproj, cfg):
    """Returns list of 8 in_maps (fp16 device layouts)."""
    N, M, C, O = cfg.N, cfg.M, cfg.C, cfg.O
    pr = perm64()
    cosq, sinq = rope_tables(freqs_cis, N)
    cosk, sink = rope_tables(freqs_cis, M)
    idx = np.concatenate([h * 64 + pr for h in range(cfg.HPC)])

    def b16(a):
        return np.ascontiguousarray(a).astype(F16NP)

    in_maps = []
    for core in range(8):
        b, j = divmod(core, 2)
        wq = Wq[j * O:(j + 1) * O, :][idx]
        wk = Wkv[j * O:(j + 1) * O, :][idx]
        wv = Wkv[C + j * O:C + (j + 1) * O, :]
        m = {
            "xT": b16(x[b].T),            # [C, N]
            "cT": b16(context[b].T),      # [C, M]
            "wqT": b16(wq.T),             # [C, O]
            "wkT": b16(wk.T),
            "wvT": b16(wv.T),
            "wpT": b16(Wproj[:, j * O:(j + 1) * O].T),  # [O, C]
            "cosq": b16(cosq), "sinq": b16(sinq),
        }
        if not (N == M):
            m["cosk"], m["sink"] = b16(cosk), b16(sink)
        in_maps.append(m)
    return in_maps


def host_gather(results, bproj, cfg):
    outs = []
    for b in range(4):
        p0 = results[2 * b]["outT"]
        p1 = results[2 * b + 1]["outT"]
        outs.append((np.asarray(p0) + np.asarray(p1)).T + bproj[None, :])
    return np.stack(outs).astype(np.float32)


def build_nc(cfg):
    N, M, C, O = cfg.N, cfg.M, cfg.C, cfg.O
    CC, OC4, HPC = cfg.CC, cfg.OC4, cfg.HPC
    NPB, NBL = cfg.NPB, cfg.NBL
    n_pb, m_pb = N // NPB, M // NPB
    MC = M // P
    NH = N // NBL
    GPI = MC // 2  # 2-mc groups per (nh, c4) iteration

    nc = bacc.Bacc("TRN2", target_bir_lowering=False, debug=False)
    xT = nc.dram_tensor("xT", [C, N], F16, kind="ExternalInput").ap()
    cT = nc.dram_tensor("cT", [C, M], F16, kind="ExternalInput").ap()
    wqT = nc.dram_tensor("wqT", [C, O], F16, kind="ExternalInput").ap()
    wkT = nc.dram_tensor("wkT", [C, O], F16, kind="ExternalInput").ap()
    wvT = nc.dram_tensor("wvT", [C, O], F16, kind="ExternalInput").ap()
    wpT = nc.dram_tensor("wpT", [O, C], F16, kind="ExternalInput").ap()
    cosq = nc.dram_tensor("cosq", [P, N], F16, kind="ExternalInput").ap()
    sinq = nc.dram_tensor("sinq", [P, N], F16, kind="ExternalInput").ap()
    if N == M:
        cosk, sink = cosq, sinq
    else:
        cosk = nc.dram_tensor("cosk", [P, M], F16, kind="ExternalInput").ap()
        sink = nc.dram_tensor("sink", [P, M], F16, kind="ExternalInput").ap()
    outT = nc.dram_tensor("outT", [C, N], F32, kind="ExternalOutput").ap()

    Exp = mybir.ActivationFunctionType.Exp
    Mult = mybir.AluOpType.mult
    Add = mybir.AluOpType.add
    dma = nc.sync.dma_start
    A_dve = EXP_A * cfg.scale

    with tile.TileContext(nc) as tc:
        with tc.tile_pool(name="persist", bufs=1) as pp:
            # ---- persistent tiles (~80 KB/partition)
            qT = pp.tile([P, OC4, N], F16, tag="qT")
            kT = pp.tile([P, OC4, M], F16, tag="kT")
            # per head 128 stationary cols: col 0 = ones (den -> psO
            # partition 0), cols 64..127 = v (attn -> psO partitions
            # 64..127; engine APs need 32-aligned partition bases).
            vAll = pp.tile([P, MC, HPC * 128], BF16, tag="vAll")
            attnT = pp.tile([P, OC4, N], F16, tag="attnT")
            nc.vector.memset(vAll[:, :, :], 1.0)

            # Attention-phase SBUF pools open FIRST so their regions are
            # disjoint from the phase-1 pools stacked above them: exp/PV
            # then carry no false deps on phase-1's last SBUF readers.
            with (
                tc.tile_pool(name="wpp", bufs=1) as wpp_pool,
                tc.tile_pool(name="exps", bufs=6) as exp_pool,
                tc.tile_pool(name="recipp", bufs=2) as recip_pool,
                tc.tile_pool(name="oev", bufs=3) as oev_pool,
            ):
                wp_sb = wpp_pool.tile([P, OC4, C], F16, tag="wp_sb")

                # ====== phase 1: K+V then Q projections, RoPE inline ======
                with (
                    tc.tile_pool(name="wqkv", bufs=1) as wqkv_pool,
                    tc.tile_pool(name="xc", bufs=2) as xc_pool,
                    tc.tile_pool(name="xq", bufs=2) as xq_pool,
                    tc.tile_pool(name="rope", bufs=2) as rope_pool,
                    tc.tile_pool(name="ps1", bufs=1, space="PSUM") as ps1,
                ):
                    wk_sb = wqkv_pool.tile([P, CC, O], F16, tag="wk_sb")
                    wv_sb = wqkv_pool.tile([P, CC, O], F16, tag="wv_sb")
                    wq_sb = wqkv_pool.tile([P, CC, O], F16, tag="wq_sb")
                    cos_q = wqkv_pool.tile([P, N], F16, tag="cos_q")
                    sin_q = wqkv_pool.tile([P, N], F16, tag="sin_q")
                    if N == M:
                        cos_k, sin_k = cos_q, sin_q
                    else:
                        cos_k = wqkv_pool.tile([P, M], F16, tag="cos_k")
                        sin_k = wqkv_pool.tile([P, M], F16, tag="sin_k")
                    wkr = wkT.rearrange("(cc p) o -> p cc o", p=P)
                    wvr = wvT.rearrange("(cc p) o -> p cc o", p=P)
                    wqr = wqT.rearrange("(cc p) o -> p cc o", p=P)

                    def rope_block(t, c4, sl, cos_t, sin_t):
                        sw = rope_pool.tile([P, NPB], F16, tag="rope_sw",
                                            name="sw")
                        nc.vector.stream_shuffle(
                            sw[:, :].bitcast(U32),
                            t[:, c4, sl].bitcast(U32), SHUFFLE_MASK)
                        t1 = rope_pool.tile([P, NPB], F16, tag="rope_t1",
                                            name="t1")
                        nc.vector.tensor_mul(t1[:, :], t[:, c4, sl],
                                             cos_t[:, sl])
                        nc.vector.tensor_mul(sw[:, :], sw[:, :],
                                             sin_t[:, sl])
                        nc.vector.tensor_add(t[:, c4, sl], t1[:, :],
                                             sw[:, :])

                    # ---- K+V projections fused (share context loads)
                    n_mc2 = NPB // P
                    for mb in range(m_pb):
                        ms = bass.ts(mb, NPB)
                        psk = [ps1.tile([P, NPB], F32, tag=f"psk{i}",
                                        name=f"psk{mb}_{i}")
                               for i in range(2)]
                        psv = [ps1.tile([P, O], F32, tag=f"psv{i}",
                                        name=f"psv{mb}_{i}")
                               for i in range(n_mc2)]
                        c_tiles = [xc_pool.tile([P, NPB], F16, tag=f"c{cc}",
                                                name=f"c{mb}_{cc}")
                                   for cc in range(CC)]
                        # mb=0: interleave weight-chunk DMAs with the
                        # context tiles so the first matmul starts early.
                        for cc in range(CC):
                            dma(c_tiles[cc][:, :], cT[bass.ts(cc, P), ms])
                            if mb == 0:
                                dma(wk_sb[:, cc, :], wkr[:, cc, :])
                                dma(wv_sb[:, cc, :], wvr[:, cc, :])
                        if mb == 0:
                            dma(cos_q[:, :], cosq)
                            dma(sin_q[:, :], sinq)
                            if N != M:
                                dma(cos_k[:, :], cosk)
                                dma(sin_k[:, :], sink)
                        for half in range(2):
                            for cc in range(CC):
                                for i in range(2):
                                    c4 = 2 * half + i
                                    nc.tensor.matmul(
                                        psk[i][:, :],
                                        wk_sb[:, cc, bass.ts(c4, P)],
                                        c_tiles[cc][:, :],
                                        start=(cc == 0), stop=(cc == CC - 1),
                                    )
                            for i in range(2):
                                c4 = 2 * half + i
                                nc.scalar.copy(kT[:, c4, ms], psk[i][:, :])
                                rope_block(kT, c4, ms, cos_k, sin_k)
                            if half == 0:
                                psk = [ps1.tile([P, NPB], F32, tag=f"psk{i}",
                                                name=f"pskb{mb}_{i}")
                                       for i in range(2)]
                        for cc in range(CC):
                            for mc2 in range(n_mc2):
                                nc.tensor.matmul(
                                    psv[mc2][:, :],
                                    c_tiles[cc][:, bass.ts(mc2, P)],
                                    wv_sb[:, cc, :],
                                    start=(cc == 0), stop=(cc == CC - 1),
                                )
                        for mc2 in range(n_mc2):
                            mc = mb * n_mc2 + mc2
                            nc.vector.tensor_copy(
                                vAll[:, mc, :].rearrange(
                                    "p (h e) -> p h e", e=128)[:, :, 64:128],
                                psv[mc2][:, :].rearrange(
                                    "p (h d) -> p h d", d=64),
                            )

                    for cc in range(CC):
                        dma(wq_sb[:, cc, :], wqr[:, cc, :])
                    dma(wp_sb[:, :, :],
                        wpT.rearrange("(oc p) e -> p oc e", p=P))

                    # ---- Q projection (2 banks, two c4-pair passes per nb)
                    for nb in range(n_pb):
                        ns = bass.ts(nb, NPB)
                        x_tiles = [xq_pool.tile([P, NPB], F16, tag=f"x{cc}",
                                                name=f"x{nb}_{cc}")
                                   for cc in range(CC)]
                        for cc in range(CC):
                            dma(x_tiles[cc][:, :], xT[bass.ts(cc, P), ns])
                        for half in range(2):
                            psq = [ps1.tile([P, NPB], F32, tag=f"psk{i}",
                                            name=f"psq{nb}_{half}_{i}")
                                   for i in range(2)]
                            for cc in range(CC):
                                for i in range(2):
                                    c4 = 2 * half + i
                                    nc.tensor.matmul(
                                        psq[i][:, :],
                                        wq_sb[:, cc, bass.ts(c4, P)],
                                        x_tiles[cc][:, :],
                                        start=(cc == 0), stop=(cc == CC - 1),
                                    )
                            for i in range(2):
                                c4 = 2 * half + i
                                nc.scalar.copy(qT[:, c4, ns], psq[i][:, :])
                                rope_block(qT, c4, ns, cos_q, sin_q)

                # ===== attention: flat pipeline over (nh, c4) + out-proj ==
                with (
                    # PSUM pool-open order fixes the bank map: psP 0-1 (on
                    # Q's banks), psO 2-3, psS 4-7 (free at KV end).
                    tc.tile_pool(name="psP", bufs=2, space="PSUM") as psP_pool,
                    tc.tile_pool(name="psO", bufs=1, space="PSUM") as psO_pool,
                    tc.tile_pool(name="psS", bufs=2, space="PSUM") as psS_pool,
                ):
                    iters = [(nh, c4) for nh in range(NH)
                             for c4 in range(OC4)]

                    def make_emitter(nh, c4):
                        nsl = bass.ts(nh, NBL)

                        def emit_group(g):
                            """S pairs for mc=2g,2g+1 (disjoint 64-row PE
                            groups, one 2-bank psS tile each) + one merged
                            [128,1024] exp per mc."""
                            out = []
                            for mc in (2 * g, 2 * g + 1):
                                msl = bass.ts(mc, P)
                                psS = psS_pool.tile(
                                    [P, 2, NBL], F32, tag="psS",
                                    name=f"psS_{nh}_{c4}_{mc}")
                                nc.tensor.matmul(
                                    psS[:, 0, :], kT[0:64, c4, msl],
                                    qT[0:64, c4, nsl],
                                    start=True, stop=True)
                                nc.tensor.matmul(
                                    psS[:, 1, :], kT[64:128, c4, msl],
                                    qT[64:128, c4, nsl],
                                    start=True, stop=True)
                                eS = exp_pool.tile(
                                    [P, 2, NBL], BF16, tag="eS",
                                    name=f"eS_{nh}_{c4}_{mc}")
                                if mc in DVE_MCS:
                                    nc.vector.tensor_scalar(
                                        eS[:, :, :].bitcast(I16),
                                        psS[:, :, :],
                                        A_dve, EXP_B, Mult, Add)
                                else:
                                    nc.scalar.activation(
                                        eS[:, :, :], psS[:, :, :], Exp,
                                        scale=cfg.scale)
                                out.append(eS)
                            return out

                        return emit_group

                    emitters = [make_emitter(nh, c4) for nh, c4 in iters]
                    esq = [emitters[0](0), emitters[0](1)]
                    for idx, (nh, c4) in enumerate(iters):
                        nsl = bass.ts(nh, NBL)
                        h1, h2 = 2 * c4, 2 * c4 + 1
                        psO1 = psO_pool.tile([P, NBL], F32, tag="psO1",
                                             name="psO1")
                        psO2 = psO_pool.tile([P, NBL], F32, tag="psO2",
                                             name="psO2")
                        for g in range(GPI):
                            if g + 2 < GPI:
                                esq.append(emitters[idx](g + 2))
                            eA, eB = esq.pop(0)
                            for j, eS in ((0, eA), (1, eB)):
                                mc = 2 * g + j
                                st, sp = (mc == 0), (mc == MC - 1)
                                nc.tensor.matmul(
                                    psO1[:, :],
                                    vAll[:, mc, bass.ds(h1 * 128, 128)],
                                    eS[:, 0, :], start=st, stop=sp)
                                nc.tensor.matmul(
                                    psO2[:, :],
                                    vAll[:, mc, bass.ds(h2 * 128, 128)],
                                    eS[:, 1, :], start=st, stop=sp)
                        # next iteration's warmup groups go ahead of the
                        # normalize so the PE stream never drains.
                        if idx + 1 < len(iters):
                            esq.append(emitters[idx + 1](0))
                            esq.append(emitters[idx + 1](1))

                        # normalize + evict (per head); den on psO part. 0.
                        # psO is released by the copy+recip pair (parallel
                        # engines, ~0.8us); the broadcast + in-place mul
                        # run later on SBUF, off the psO critical path, so
                        # the next iteration's first PV is not stalled.
                        for hh, psO in ((0, psO1), (1, psO2)):
                            rows = slice(hh * 64, hh * 64 + 64)
                            rc = recip_pool.tile([1, NBL], F32,
                                                 tag=f"rc{hh}", name="rc")
                            nc.vector.reciprocal_approx_fast(rc[:, :],
                                                             psO[0:1, :])
                            nc.scalar.copy(attnT[rows, c4, nsl],
                                           psO[64:128, :])
                            rb = recip_pool.tile([P, NBL], F32,
                                                 tag=f"rb{hh}", name="rb")
                            nc.gpsimd.partition_broadcast(rb[:, :],
                                                          rc[:, :])
                            nc.vector.tensor_mul(attnT[rows, c4, nsl],
                                                 attnT[rows, c4, nsl],
                                                 rb[rows, :])

                        # ---- out-proj for this nh (overlaps nh+1)
                        if c4 == OC4 - 1:
                            for ec in range(C // P):
                                ps = psP_pool.tile([P, NBL], F32,
                                                   tag="ps_out")
                                for oc in range(OC4):
                                    nc.tensor.matmul(
                                        ps[:, :],
                                        wp_sb[:, oc, bass.ts(ec, P)],
                                        attnT[:, oc, nsl],
                                        start=(oc == 0),
                                        stop=(oc == OC4 - 1),
                                    )
                                ot = oev_pool.tile([P, NBL], F32, tag="ot")
                                # high priority: the evict must not queue
                                # behind exp ops or psP recycling starves
                                # and out-proj MMs dribble into the S
                                # bursts (tiling-mode thrash).
                                with tc.high_priority():
                                    if ec % 2 == 0:
                                        nc.scalar.copy(ot[:, :], ps[:, :])
                                    else:
                                        nc.vector.tensor_copy(ot[:, :],
                                                              ps[:, :])
                                dma(outT[bass.ts(ec, P), nsl], ot[:, :])

    nc.compile()
    return nc


# ---------------------------------------------------------------- emulation
def _bf(a):
    return np.asarray(a).astype(BFNP).astype(np.float32)


def _f16(a):
    return np.asarray(a).astype(np.float16).astype(np.float32)


def _schraudolph(x):
    """Emulate the DVE bf16 exp trick (truncating f32->i16 cast)."""
    i = np.floor(EXP_A * x + EXP_B).astype(np.int16)
    return i.view(BFNP).astype(np.float32)


def emulate_core(m, cfg):
    """Numpy replica of the device program (layout + numerics validation)."""
    N, M, C, O = cfg.N, cfg.M, cfg.C, cfg.O
    xT = _f16(m["xT"])
    cT = _f16(m["cT"])
    qT = _f16(_f16(m["wqT"]).T @ xT)
    kT = _f16(_f16(m["wkT"]).T @ cT)
    v = _bf(cT.T @ _f16(m["wvT"]))
    cosk = _f16(m.get("cosk", m["cosq"]))
    sink = _f16(m.get("sink", m["sinq"]))
    cosq_t, sinq_t = _f16(m["cosq"]), _f16(m["sinq"])

    def rope(tT, cos, sin, L):
        t = tT.reshape(cfg.OC4, P, L)
        out = np.empty_like(t)
        for c4 in range(cfg.OC4):
            blk = t[c4]
            sw = np.empty_like(blk)
            for s in range(4):
                for i in range(32):
                    sw[s * 32 + i] = blk[s * 32 + SHUFFLE_MASK[i]]
            out[c4] = _f16(_f16(blk * cos) + _f16(sw * sin))
        return out.reshape(O, L)

    qT = rope(qT, cosq_t, sinq_t, N)
    kT = rope(kT, cosk, sink, M)

    attnT = np.empty((O, N), np.float32)
    for h in range(cfg.HPC):
        qh = qT[h * 64:(h + 1) * 64, :]
        kh = kT[h * 64:(h + 1) * 64, :]
        S = kh.T @ qh
        E = np.empty((M, N), np.float32)
        for mc in range(M // P):
            sl = slice(mc * P, (mc + 1) * P)
            if mc in DVE_MCS:
                E[sl] = _schraudolph(cfg.scale * S[sl])
            else:
                E[sl] = _bf(np.exp(cfg.scale * S[sl]))
        vh = v[:, h * 64:(h + 1) * 64]
        num = vh.T @ E
        den = E.sum(axis=0)
        attnT[h * 64:(h + 1) * 64] = _f16(num / den[None, :])
    return _f16(m["wpT"]).T.astype(np.float32) @ attnT


# ---------------------------------------------------------------- driver
_NC_CACHE = {}


def _get_nc(cfg):
    key = (cfg.N, cfg.M)
    if key not in _NC_CACHE:
        _NC_CACHE[key] = build_nc(cfg)
    return _NC_CACHE[key]


def _run(inputs, trace=False):
    cfg = CFG()
    nc = _get_nc(cfg)
    in_maps = host_prep(
        np.asarray(inputs["x"], np.float32),
        np.asarray(inputs["context"], np.float32),
        np.asarray(inputs["freqs_cis"], np.float32),
        np.asarray(inputs["Wq"], np.float32),
        np.asarray(inputs["Wkv"], np.float32),
        np.asarray(inputs["Wproj"], np.float32),
        cfg,
    )
    res = run_bass_kernel_spmd(nc, in_maps, list(range(8)), trace=trace)
    out = host_gather(res.results, np.asarray(inputs["bproj"], np.float32), cfg)
    return out, res


def kernel(**inputs):
    out, _ = _run(inputs, trace=False)
    return out


def timed_run(inputs):
    _, res = _run(inputs, trace=True)
    return res.exec_time_ns, res


# revision 18
# speedup vs baseline: 1.0455x; 1.0455x over previous
"""Cross-attention Trainium2 kernel: build, host prep/gather, emulation.

Sharding: 8 cores = 4 batches x 2 head-halves. Core c=(b,j) computes
heads j*8..j*8+8 for batch b, producing a partial out.T [C, N]; host
sums the two partials per batch and adds bias.

All matmuls run in fp16 (PSUM accumulation f32). Contraction dims sit
on SBUF partitions via host-side transposes:
  kT[o,m] = wkT.T @ cT ; v[m,o] = cT.T @ wvT ; qT[o,n] = wqT.T @ xT
  RoPE: pair-partner lives 16 partitions away inside each 32-partition
    quadrant (host permutes W columns accordingly) so one DVE
    stream_shuffle (on a uint32 view) fetches it; q' = q*cos + shuf(q)*sin.
    RoPE runs INSIDE phase 1, per 512-block, right after each eviction,
    so the DVE is free for exp during attention.
  Phase 1 order: K+V proj (fused, shared context loads) then Q proj
    (2 PSUM banks, two c4-pair passes) so attention can overlap the Q
    tail. Attention SBUF pools open BEFORE phase-1 pools (disjoint
    regions -> no false deps); PSUM map: ps1 psk 0-1/psv 2-5, then
    psP 0-1 (Q's banks, freed last), psO 2-3, psS 4-7 (free at KV end).
  Attention: one flat software pipeline over (nh, c4) iterations.
    S.T tile [m,n] = kT_h.T @ qT_h  (K=64, NBL=512 wide); the two heads'
    S matmuls land in ONE two-bank PSUM tile psS[:, h, :] and run
    concurrently on disjoint 64-row PE groups (tile_position auto).
    S pairs are emitted in 2-mc batches so the 64-row<->128-row tiling
    mode switch (which drains the PE array) happens twice per 2 mc.
    The next iteration's first two S/exp groups are emitted at the tail
    of the current iteration, so the PE never waits on the exp warmup.
    exp: ONE op per mc covering both heads [128, 1024] (the psS tile
    spans 2 adjacent PSUM banks), alternating engines by mc: ACT Exp
    activation vs DVE Schraudolph bit-trick writing bf16 bits through
    an int16 cast: bf16_bits(e^x) ~= int16(184.665*x + 16250.75).
  eS/vAll are bf16 (exp spans e^-13..e^13 -- fp16 would overflow);
    the q/k/S path and attnT/wp are fp16 for mantissa precision.
  PV stationary per head is 128 wide: col 0 = ones (softmax denominator
    lands on psO partition 0, where reciprocal_approx_fast can read it
    directly), cols 64..127 = v (attn rows at 64-aligned partition base;
    engine APs require 32-aligned bases). psO[p,n] += [1|..|v_h].T @ expS.
  attnT = psO[64:128] * bcast(recip(psO[0])) on DVE/gpsimd.
  outT[e, n] = wpT.T @ attnT, emitted per nh right after attention(nh)
    so out-proj matmuls/DMAs overlap attention(nh+1).
"""

import sys

sys.path.insert(0, "/opt/trn_rl_repo")

import numpy as np
import ml_dtypes

import concourse.bass as bass
import concourse.tile as tile
from concourse import bacc, mybir
from concourse.bass_utils import run_bass_kernel_spmd

P = 128
SHUFFLE_MASK = [(i + 16) % 32 for i in range(32)]
F32 = mybir.dt.float32
F16 = mybir.dt.float16
BF16 = mybir.dt.bfloat16
I16 = mybir.dt.int16
U32 = mybir.dt.uint32
F16NP = np.float16
BFNP = ml_dtypes.bfloat16

# bf16 Schraudolph: bf16_bits(e^x) ~= int16(EXP_A*x + EXP_B)
# (bf16 target: exponent covers |x| up to ~30 -- scores reach |x|~13)
EXP_A = 184.6649652337873
EXP_B = 16250.75
# mc chunks whose exp (both heads) runs on DVE via Schraudolph; the rest
# on ACT. 7/16 on DVE: DVE also carries the normalize ops.
DVE_MCS = frozenset({1, 3, 5, 7, 9, 11, 13})


class CFG:
    def __init__(self, N=2048, M=2048):
        self.N, self.M = N, M
        self.C = 1024
        self.H = 16
        self.D = 64
        self.O = 512           # local head dim total (8 heads x 64)
        self.CC = self.C // P  # 8 c-chunks
        self.OC4 = self.O // P  # 4 o-chunks
        self.HPC = 8           # heads per core
        self.NPB = min(512, N)   # proj n-block
        self.NBL = min(512, N)   # attention n-block
        self.scale = self.D ** -0.5


def perm64():
    """Device partition row p (within a head's 64) -> original component."""
    out = []
    for p in range(64):
        q2, i = divmod(p, 32)
        pair = q2 * 16 + (i % 16)
        out.append(2 * pair + (0 if i < 16 else 1))
    return np.array(out)


def rope_tables(fc, L):
    """cos/sin tables [128, L] matching the permuted q/k layout."""
    cos = np.empty((P, L), np.float32)
    sin = np.empty((P, L), np.float32)
    for p in range(P):
        p64 = p % 64
        pair = (p64 // 32) * 16 + (p64 % 16)
        is_even = (p64 % 32) < 16
        cos[p] = fc[:L, pair, 0]
        sin[p] = fc[:L, pair, 1] * (-1.0 if is_even else 1.0)
    return cos, sin


def host_prep(x, context, freqs_cis, Wq, Wkv, Wproj, cfg):
    """Returns list of 8 in_maps (fp16 device layouts)."""
    N, M, C, O = cfg.N, cfg.M, cfg.C, cfg.O
    pr = perm64()
    cosq, sinq = rope_tables(freqs_cis, N)
    cosk, sink = rope_tables(freqs_cis, M)
    idx = np.concatenate([h * 64 + pr for h in range(cfg.HPC)])

    def b16(a):
        return np.ascontiguousarray(a).astype(F16NP)

    in_maps = []
    for core in range(8):
        b, j = divmod(core, 2)
        wq = Wq[j * O:(j + 1) * O, :][idx]
        wk = Wkv[j * O:(j + 1) * O, :][idx]
        wv = Wkv[C + j * O:C + (j + 1) * O, :]
        m = {
            "xT": b16(x[b].T),            # [C, N]
            "cT": b16(context[b].T),      # [C, M]
            "wqT": b16(wq.T),             # [C, O]
            "wkT": b16(wk.T),
            "wvT": b16(wv.T),
            "wpT": b16(Wproj[:, j * O:(j + 1) * O].T),  # [O, C]
            "cosq": b16(cosq), "sinq": b16(sinq),
        }
        if not (N == M):
            m["cosk"], m["sink"] = b16(cosk), b16(sink)
        in_maps.append(m)
    return in_maps


def host_gather(results, bproj, cfg):
    outs = []
    for b in range(4):
        p0 = results[2 * b]["outT"]
        p1 = results[2 * b + 1]["outT"]
        outs.append((np.asarray(p0) + np.asarray(p1)).T + bproj[None, :])
    return np.stack(outs).astype(np.float32)


def build_nc(cfg):
    N, M, C, O = cfg.N, cfg.M, cfg.C, cfg.O
    CC, OC4, HPC = cfg.CC, cfg.OC4, cfg.HPC
    NPB, NBL = cfg.NPB, cfg.NBL
    n_pb, m_pb = N // NPB, M // NPB
    MC = M // P
    NH = N // NBL
    GPI = MC // 2  # 2-mc groups per (nh, c4) iteration

    nc = bacc.Bacc("TRN2", target_bir_lowering=False, debug=False)
    xT = nc.dram_tensor("xT", [C, N], F16, kind="ExternalInput").ap()
    cT = nc.dram_tensor("cT", [C, M], F16, kind="ExternalInput").ap()
    wqT = nc.dram_tensor("wqT", [C, O], F16, kind="ExternalInput").ap()
    wkT = nc.dram_tensor("wkT", [C, O], F16, kind="ExternalInput").ap()
    wvT = nc.dram_tensor("wvT", [C, O], F16, kind="ExternalInput").ap()
    wpT = nc.dram_tensor("wpT", [O, C], F16, kind="ExternalInput").ap()
    cosq = nc.dram_tensor("cosq", [P, N], F16, kind="ExternalInput").ap()
    sinq = nc.dram_tensor("sinq", [P, N], F16, kind="ExternalInput").ap()
    if N == M:
        cosk, sink = cosq, sinq
    else:
        cosk = nc.dram_tensor("cosk", [P, M], F16, kind="ExternalInput").ap()
        sink = nc.dram_tensor("sink", [P, M], F16, kind="ExternalInput").ap()
    outT = nc.dram_tensor("outT", [C, N], F32, kind="ExternalOutput").ap()

    Exp = mybir.ActivationFunctionType.Exp
    Copy = mybir.ActivationFunctionType.Copy
    Mult = mybir.AluOpType.mult
    Add = mybir.AluOpType.add
    dma = nc.sync.dma_start
    A_dve = EXP_A * cfg.scale

    with tile.TileContext(nc) as tc:
        with tc.tile_pool(name="persist", bufs=1) as pp:
            # ---- persistent tiles (~80 KB/partition)
            qT = pp.tile([P, OC4, N], F16, tag="qT")
            kT = pp.tile([P, OC4, M], F16, tag="kT")
            # per head 128 stationary cols: col 0 = ones (den -> psO
            # partition 0), cols 64..127 = v (attn -> psO partitions
            # 64..127; engine APs need 32-aligned partition bases).
            vAll = pp.tile([P, MC, HPC * 128], BF16, tag="vAll")
            attnT = pp.tile([P, OC4, N], F16, tag="attnT")
            # ones column holds 1/128: psO[0] = den/128, and the psO
            # eviction scales by 1/128 so the un-normalized fp16 copy
            # cannot overflow (raw num reaches ~2e6 > fp16 max).
            nc.vector.memset(vAll[:, :, :], 1.0 / 128.0)

            # Attention-phase SBUF pools open FIRST so their regions are
            # disjoint from the phase-1 pools stacked above them: exp/PV
            # then carry no false deps on phase-1's last SBUF readers.
            with (
                tc.tile_pool(name="wpp", bufs=1) as wpp_pool,
                tc.tile_pool(name="exps", bufs=6) as exp_pool,
                tc.tile_pool(name="recipp", bufs=2) as recip_pool,
                tc.tile_pool(name="oev", bufs=3) as oev_pool,
            ):
                wp_sb = wpp_pool.tile([P, OC4, C], F16, tag="wp_sb")

                # ====== phase 1: K+V then Q projections, RoPE inline ======
                with (
                    tc.tile_pool(name="wqkv", bufs=1) as wqkv_pool,
                    tc.tile_pool(name="xc", bufs=2) as xc_pool,
                    tc.tile_pool(name="xq", bufs=2) as xq_pool,
                    tc.tile_pool(name="rope", bufs=2) as rope_pool,
                    tc.tile_pool(name="ps1", bufs=1, space="PSUM") as ps1,
                ):
                    wk_sb = wqkv_pool.tile([P, CC, O], F16, tag="wk_sb")
                    wv_sb = wqkv_pool.tile([P, CC, O], F16, tag="wv_sb")
                    wq_sb = wqkv_pool.tile([P, CC, O], F16, tag="wq_sb")
                    cos_q = wqkv_pool.tile([P, N], F16, tag="cos_q")
                    sin_q = wqkv_pool.tile([P, N], F16, tag="sin_q")
                    if N == M:
                        cos_k, sin_k = cos_q, sin_q
                    else:
                        cos_k = wqkv_pool.tile([P, M], F16, tag="cos_k")
                        sin_k = wqkv_pool.tile([P, M], F16, tag="sin_k")
                    wkr = wkT.rearrange("(cc p) o -> p cc o", p=P)
                    wvr = wvT.rearrange("(cc p) o -> p cc o", p=P)
                    wqr = wqT.rearrange("(cc p) o -> p cc o", p=P)

                    def rope_block(t, c4, sl, cos_t, sin_t):
                        sw = rope_pool.tile([P, NPB], F16, tag="rope_sw",
                                            name="sw")
                        nc.vector.stream_shuffle(
                            sw[:, :].bitcast(U32),
                            t[:, c4, sl].bitcast(U32), SHUFFLE_MASK)
                        t1 = rope_pool.tile([P, NPB], F16, tag="rope_t1",
                                            name="t1")
                        nc.vector.tensor_mul(t1[:, :], t[:, c4, sl],
                                             cos_t[:, sl])
                        nc.vector.tensor_mul(sw[:, :], sw[:, :],
                                             sin_t[:, sl])
                        nc.vector.tensor_add(t[:, c4, sl], t1[:, :],
                                             sw[:, :])

                    # ---- K+V projections fused (share context loads)
                    n_mc2 = NPB // P
                    for mb in range(m_pb):
                        ms = bass.ts(mb, NPB)
                        psk = [ps1.tile([P, NPB], F32, tag=f"psk{i}",
                                        name=f"psk{mb}_{i}")
                               for i in range(2)]
                        psv = [ps1.tile([P, O], F32, tag=f"psv{i}",
                                        name=f"psv{mb}_{i}")
                               for i in range(n_mc2)]
                        c_tiles = [xc_pool.tile([P, NPB], F16, tag=f"c{cc}",
                                                name=f"c{mb}_{cc}")
                                   for cc in range(CC)]
                        # mb=0: interleave weight-chunk DMAs with the
                        # context tiles so the first matmul starts early.
                        for cc in range(CC):
                            dma(c_tiles[cc][:, :], cT[bass.ts(cc, P), ms])
                            if mb == 0:
                                dma(wk_sb[:, cc, :], wkr[:, cc, :])
                                dma(wv_sb[:, cc, :], wvr[:, cc, :])
                        if mb == 0:
                            dma(cos_q[:, :], cosq)
                            dma(sin_q[:, :], sinq)
                            if N != M:
                                dma(cos_k[:, :], cosk)
                                dma(sin_k[:, :], sink)
                        for half in range(2):
                            for cc in range(CC):
                                for i in range(2):
                                    c4 = 2 * half + i
                                    nc.tensor.matmul(
                                        psk[i][:, :],
                                        wk_sb[:, cc, bass.ts(c4, P)],
                                        c_tiles[cc][:, :],
                                        start=(cc == 0), stop=(cc == CC - 1),
                                    )
                            for i in range(2):
                                c4 = 2 * half + i
                                nc.scalar.copy(kT[:, c4, ms], psk[i][:, :])
                                rope_block(kT, c4, ms, cos_k, sin_k)
                            if half == 0:
                                psk = [ps1.tile([P, NPB], F32, tag=f"psk{i}",
                                                name=f"pskb{mb}_{i}")
                                       for i in range(2)]
                        for cc in range(CC):
                            for mc2 in range(n_mc2):
                                nc.tensor.matmul(
                                    psv[mc2][:, :],
                                    c_tiles[cc][:, bass.ts(mc2, P)],
                                    wv_sb[:, cc, :],
                                    start=(cc == 0), stop=(cc == CC - 1),
                                )
                        for mc2 in range(n_mc2):
                            mc = mb * n_mc2 + mc2
                            nc.vector.tensor_copy(
                                vAll[:, mc, :].rearrange(
                                    "p (h e) -> p h e", e=128)[:, :, 64:128],
                                psv[mc2][:, :].rearrange(
                                    "p (h d) -> p h d", d=64),
                            )

                    for cc in range(CC):
                        dma(wq_sb[:, cc, :], wqr[:, cc, :])
                    dma(wp_sb[:, :, :],
                        wpT.rearrange("(oc p) e -> p oc e", p=P))

                    # ---- Q projection (2 banks, two c4-pair passes per nb)
                    for nb in range(n_pb):
                        ns = bass.ts(nb, NPB)
                        x_tiles = [xq_pool.tile([P, NPB], F16, tag=f"x{cc}",
                                                name=f"x{nb}_{cc}")
                                   for cc in range(CC)]
                        for cc in range(CC):
                            dma(x_tiles[cc][:, :], xT[bass.ts(cc, P), ns])
                        for half in range(2):
                            psq = [ps1.tile([P, NPB], F32, tag=f"psk{i}",
                                            name=f"psq{nb}_{half}_{i}")
                                   for i in range(2)]
                            for cc in range(CC):
                                for i in range(2):
                                    c4 = 2 * half + i
                                    nc.tensor.matmul(
                                        psq[i][:, :],
                                        wq_sb[:, cc, bass.ts(c4, P)],
                                        x_tiles[cc][:, :],
                                        start=(cc == 0), stop=(cc == CC - 1),
                                    )
                            for i in range(2):
                                c4 = 2 * half + i
                                nc.scalar.copy(qT[:, c4, ns], psq[i][:, :])
                                rope_block(qT, c4, ns, cos_q, sin_q)

                # ===== attention: flat pipeline over (nh, c4) + out-proj ==
                with (
                    # PSUM pool-open order fixes the bank map: psP 0-1 (on
                    # Q's banks), psO 2-3, psS 4-7 (free at KV end).
                    tc.tile_pool(name="psP", bufs=2, space="PSUM") as psP_pool,
                    tc.tile_pool(name="psO", bufs=1, space="PSUM") as psO_pool,
                    tc.tile_pool(name="psS", bufs=2, space="PSUM") as psS_pool,
                ):
                    iters = [(nh, c4) for nh in range(NH)
                             for c4 in range(OC4)]

                    def make_emitter(nh, c4):
                        nsl = bass.ts(nh, NBL)

                        def emit_group(g):
                            """S pairs for mc=2g,2g+1 (disjoint 64-row PE
                            groups, one 2-bank psS tile each) + one merged
                            [128,1024] exp per mc."""
                            out = []
                            for mc in (2 * g, 2 * g + 1):
                                msl = bass.ts(mc, P)
                                psS = psS_pool.tile(
                                    [P, 2, NBL], F32, tag="psS",
                                    name=f"psS_{nh}_{c4}_{mc}")
                                nc.tensor.matmul(
                                    psS[:, 0, :], kT[0:64, c4, msl],
                                    qT[0:64, c4, nsl],
                                    start=True, stop=True)
                                nc.tensor.matmul(
                                    psS[:, 1, :], kT[64:128, c4, msl],
                                    qT[64:128, c4, nsl],
                                    start=True, stop=True)
                                eS = exp_pool.tile(
                                    [P, 2, NBL], BF16, tag="eS",
                                    name=f"eS_{nh}_{c4}_{mc}")
                                if mc in DVE_MCS:
                                    nc.vector.tensor_scalar(
                                        eS[:, :, :].bitcast(I16),
                                        psS[:, :, :],
                                        A_dve, EXP_B, Mult, Add)
                                else:
                                    nc.scalar.activation(
                                        eS[:, :, :], psS[:, :, :], Exp,
                                        scale=cfg.scale)
                                out.append(eS)
                            return out

                        return emit_group

                    emitters = [make_emitter(nh, c4) for nh, c4 in iters]
                    esq = [emitters[0](0), emitters[0](1)]
                    for idx, (nh, c4) in enumerate(iters):
                        nsl = bass.ts(nh, NBL)
                        h1, h2 = 2 * c4, 2 * c4 + 1
                        psO1 = psO_pool.tile([P, NBL], F32, tag="psO1",
                                             name="psO1")
                        psO2 = psO_pool.tile([P, NBL], F32, tag="psO2",
                                             name="psO2")
                        for g in range(GPI):
                            if g + 2 < GPI:
                                esq.append(emitters[idx](g + 2))
                            eA, eB = esq.pop(0)
                            for j, eS in ((0, eA), (1, eB)):
                                mc = 2 * g + j
                                st, sp = (mc == 0), (mc == MC - 1)
                                nc.tensor.matmul(
                                    psO1[:, :],
                                    vAll[:, mc, bass.ds(h1 * 128, 128)],
                                    eS[:, 0, :], start=st, stop=sp)
                                nc.tensor.matmul(
                                    psO2[:, :],
                                    vAll[:, mc, bass.ds(h2 * 128, 128)],
                                    eS[:, 1, :], start=st, stop=sp)
                        # next iteration's warmup groups go ahead of the
                        # normalize so the PE stream never drains.
                        if idx + 1 < len(iters):
                            esq.append(emitters[idx + 1](0))
                            esq.append(emitters[idx + 1](1))

                        # normalize + evict (per head); den on psO part. 0.
                        # psO is released by the copy+recip pair (parallel
                        # engines, ~0.8us); the broadcast + in-place mul
                        # run later on SBUF, off the psO critical path, so
                        # the next iteration's first PV is not stalled.
                        for hh, psO in ((0, psO1), (1, psO2)):
                            rows = slice(hh * 64, hh * 64 + 64)
                            rc = recip_pool.tile([1, NBL], F32,
                                                 tag=f"rc{hh}", name="rc")
                            nc.vector.reciprocal_approx_fast(rc[:, :],
                                                             psO[0:1, :])
                            nc.scalar.activation(attnT[rows, c4, nsl],
                                                 psO[64:128, :], Copy,
                                                 scale=1.0 / 128.0)
                            rb = recip_pool.tile([P, NBL], F32,
                                                 tag=f"rb{hh}", name="rb")
                            nc.gpsimd.partition_broadcast(rb[:, :],
                                                          rc[:, :])
                            nc.vector.tensor_mul(attnT[rows, c4, nsl],
                                                 attnT[rows, c4, nsl],
                                                 rb[rows, :])

                        # ---- out-proj for this nh (overlaps nh+1)
                        if c4 == OC4 - 1:
                            for ec in range(C // P):
                                ps = psP_pool.tile([P, NBL], F32,
                                                   tag="ps_out")
                                for oc in range(OC4):
                                    nc.tensor.matmul(
                                        ps[:, :],
                                        wp_sb[:, oc, bass.ts(ec, P)],
                                        attnT[:, oc, nsl],
                                        start=(oc == 0),
                                        stop=(oc == OC4 - 1),
                                    )
                                ot = oev_pool.tile([P, NBL], F32, tag="ot")
                                if ec % 2 == 0:
                                    nc.scalar.copy(ot[:, :], ps[:, :])
                                else:
                                    nc.vector.tensor_copy(ot[:, :],
                                                          ps[:, :])
                                dma(outT[bass.ts(ec, P), nsl], ot[:, :])

    nc.compile()
    return nc


# ---------------------------------------------------------------- emulation
def _bf(a):
    return np.asarray(a).astype(BFNP).astype(np.float32)


def _f16(a):
    return np.asarray(a).astype(np.float16).astype(np.float32)


def _schraudolph(x):
    """Emulate the DVE bf16 exp trick (truncating f32->i16 cast)."""
    i = np.floor(EXP_A * x + EXP_B).astype(np.int16)
    return i.view(BFNP).astype(np.float32)


def emulate_core(m, cfg):
    """Numpy replica of the device program (layout + numerics validation)."""
    N, M, C, O = cfg.N, cfg.M, cfg.C, cfg.O
    xT = _f16(m["xT"])
    cT = _f16(m["cT"])
    qT = _f16(_f16(m["wqT"]).T @ xT)
    kT = _f16(_f16(m["wkT"]).T @ cT)
    v = _bf(cT.T @ _f16(m["wvT"]))
    cosk = _f16(m.get("cosk", m["cosq"]))
    sink = _f16(m.get("sink", m["sinq"]))
    cosq_t, sinq_t = _f16(m["cosq"]), _f16(m["sinq"])

    def rope(tT, cos, sin, L):
        t = tT.reshape(cfg.OC4, P, L)
        out = np.empty_like(t)
        for c4 in range(cfg.OC4):
            blk = t[c4]
            sw = np.empty_like(blk)
            for s in range(4):
                for i in range(32):
                    sw[s * 32 + i] = blk[s * 32 + SHUFFLE_MASK[i]]
            out[c4] = _f16(_f16(blk * cos) + _f16(sw * sin))
        return out.reshape(O, L)

    qT = rope(qT, cosq_t, sinq_t, N)
    kT = rope(kT, cosk, sink, M)

    attnT = np.empty((O, N), np.float32)
    for h in range(cfg.HPC):
        qh = qT[h * 64:(h + 1) * 64, :]
        kh = kT[h * 64:(h + 1) * 64, :]
        S = kh.T @ qh
        E = np.empty((M, N), np.float32)
        for mc in range(M // P):
            sl = slice(mc * P, (mc + 1) * P)
            if mc in DVE_MCS:
                E[sl] = _schraudolph(cfg.scale * S[sl])
            else:
                E[sl] = _bf(np.exp(cfg.scale * S[sl]))
        vh = v[:, h * 64:(h + 1) * 64]
        num = vh.T @ E
        den = E.sum(axis=0)
        # device: fp16 copy of num/128, then * (128/den)
        attnT[h * 64:(h + 1) * 64] = _f16(
            _f16(num / 128.0) * (128.0 / den)[None, :])
    return _f16(m["wpT"]).T.astype(np.float32) @ attnT


# ---------------------------------------------------------------- driver
_NC_CACHE = {}


def _get_nc(cfg):
    key = (cfg.N, cfg.M)
    if key not in _NC_CACHE:
        _NC_CACHE[key] = build_nc(cfg)
    return _NC_CACHE[key]


def _run(inputs, trace=False):
    cfg = CFG()
    nc = _get_nc(cfg)
    in_maps = host_prep(
        np.asarray(inputs["x"], np.float32),
        np.asarray(inputs["context"], np.float32),
        np.asarray(inputs["freqs_cis"], np.float32),
        np.asarray(inputs["Wq"], np.float32),
        np.asarray(inputs["Wkv"], np.float32),
        np.asarray(inputs["Wproj"], np.float32),
        cfg,
    )
    res = run_bass_kernel_spmd(nc, in_maps, list(range(8)), trace=trace)
    out = host_gather(res.results, np.asarray(inputs["bproj"], np.float32), cfg)
    return out, res


def kernel(**inputs):
    out, _ = _run(inputs, trace=False)
    return out


def timed_run(inputs):
    _, res = _run(inputs, trace=True)
    return res.exec_time_ns, res


# revision 19
# speedup vs baseline: 1.0902x; 1.0427x over previous
"""Cross-attention Trainium2 kernel: build, host prep/gather, emulation.

Sharding: 8 cores = 4 batches x 2 head-halves. Core c=(b,j) computes
heads j*8..j*8+8 for batch b, producing a partial out.T [C, N]; host
sums the two partials per batch and adds bias.

All matmuls run in fp16 (PSUM accumulation f32). Contraction dims sit
on SBUF partitions via host-side transposes:
  kT[o,m] = wkT.T @ cT ; v[m,o] = cT.T @ wvT ; qT[o,n] = wqT.T @ xT
  RoPE: pair-partner lives 16 partitions away inside each 32-partition
    quadrant (host permutes W columns accordingly) so one DVE
    stream_shuffle (on a uint32 view) fetches it; q' = q*cos + shuf(q)*sin.
    RoPE runs INSIDE phase 1, per 512-block, right after each eviction,
    so the DVE is free for exp during attention.
  Phase 1 order: K+V proj (fused, shared context loads) then Q proj
    (2 PSUM banks, two c4-pair passes) so attention can overlap the Q
    tail. Attention SBUF pools open BEFORE phase-1 pools (disjoint
    regions -> no false deps); PSUM map: ps1 psk 0-1/psv 2-5, then
    psP 0-1 (Q's banks, freed last), psO 2-3, psS 4-7 (free at KV end).
  Attention: one flat software pipeline over (nh, c4) iterations.
    S.T tile [m,n] = kT_h.T @ qT_h  (K=64, NBL=512 wide); the two heads'
    S matmuls land in ONE two-bank PSUM tile psS[:, h, :] and run
    concurrently on disjoint 64-row PE groups (tile_position auto).
    S pairs are emitted in 2-mc batches so the 64-row<->128-row tiling
    mode switch (which drains the PE array) happens twice per 2 mc.
    The next iteration's first two S/exp groups are emitted at the tail
    of the current iteration, so the PE never waits on the exp warmup.
    exp: ONE op per mc covering both heads [128, 1024] (the psS tile
    spans 2 adjacent PSUM banks), alternating engines by mc: ACT Exp
    activation vs DVE Schraudolph bit-trick writing bf16 bits through
    an int16 cast: bf16_bits(e^x) ~= int16(184.665*x + 16250.75).
  eS/vAll are bf16 (exp spans e^-13..e^13 -- fp16 would overflow);
    the q/k/S path and attnT/wp are fp16 for mantissa precision.
  PV stationary per head is 128 wide: col 0 = ones (softmax denominator
    lands on psO partition 0, where reciprocal_approx_fast can read it
    directly), cols 64..127 = v (attn rows at 64-aligned partition base;
    engine APs require 32-aligned bases). psO[p,n] += [1|..|v_h].T @ expS.
  attnT = psO[64:128] * bcast(recip(psO[0])) on DVE/gpsimd.
  outT[e, n] = wpT.T @ attnT, emitted per nh right after attention(nh)
    so out-proj matmuls/DMAs overlap attention(nh+1).
"""

import sys

sys.path.insert(0, "/opt/trn_rl_repo")

import numpy as np
import ml_dtypes

import concourse.bass as bass
import concourse.tile as tile
from concourse import bacc, mybir
from concourse.bass_utils import run_bass_kernel_spmd

P = 128
SHUFFLE_MASK = [(i + 16) % 32 for i in range(32)]
F32 = mybir.dt.float32
F16 = mybir.dt.float16
BF16 = mybir.dt.bfloat16
I16 = mybir.dt.int16
U32 = mybir.dt.uint32
F16NP = np.float16
BFNP = ml_dtypes.bfloat16

# bf16 Schraudolph: bf16_bits(e^x) ~= int16(EXP_A*x + EXP_B)
# (bf16 target: exponent covers |x| up to ~30 -- scores reach |x|~13)
EXP_A = 184.6649652337873
EXP_B = 16250.75
# mc chunks whose exp (both heads) runs on DVE via Schraudolph; the rest
# on ACT. 7/16 on DVE: DVE also carries the normalize ops.
DVE_MCS = frozenset({1, 3, 5, 7, 9, 11, 13})


class CFG:
    def __init__(self, N=2048, M=2048):
        self.N, self.M = N, M
        self.C = 1024
        self.H = 16
        self.D = 64
        self.O = 512           # local head dim total (8 heads x 64)
        self.CC = self.C // P  # 8 c-chunks
        self.OC4 = self.O // P  # 4 o-chunks
        self.HPC = 8           # heads per core
        self.NPB = min(512, N)   # proj n-block
        self.NBL = min(512, N)   # attention n-block
        self.scale = self.D ** -0.5


def perm64():
    """Device partition row p (within a head's 64) -> original component."""
    out = []
    for p in range(64):
        q2, i = divmod(p, 32)
        pair = q2 * 16 + (i % 16)
        out.append(2 * pair + (0 if i < 16 else 1))
    return np.array(out)


def rope_tables(fc, L):
    """cos/sin tables [128, L] matching the permuted q/k layout."""
    cos = np.empty((P, L), np.float32)
    sin = np.empty((P, L), np.float32)
    for p in range(P):
        p64 = p % 64
        pair = (p64 // 32) * 16 + (p64 % 16)
        is_even = (p64 % 32) < 16
        cos[p] = fc[:L, pair, 0]
        sin[p] = fc[:L, pair, 1] * (-1.0 if is_even else 1.0)
    return cos, sin


def host_prep(x, context, freqs_cis, Wq, Wkv, Wproj, cfg):
    """Returns list of 8 in_maps (fp16 device layouts)."""
    N, M, C, O = cfg.N, cfg.M, cfg.C, cfg.O
    pr = perm64()
    cosq, sinq = rope_tables(freqs_cis, N)
    cosk, sink = rope_tables(freqs_cis, M)
    idx = np.concatenate([h * 64 + pr for h in range(cfg.HPC)])

    def b16(a):
        return np.ascontiguousarray(a).astype(F16NP)

    in_maps = []
    for core in range(8):
        b, j = divmod(core, 2)
        wq = Wq[j * O:(j + 1) * O, :][idx]
        wk = Wkv[j * O:(j + 1) * O, :][idx]
        wv = Wkv[C + j * O:C + (j + 1) * O, :]
        m = {
            "xT": b16(x[b].T),            # [C, N]
            "cT": b16(context[b].T),      # [C, M]
            "wqT": b16(wq.T),             # [C, O]
            "wkT": b16(wk.T),
            "wvT": b16(wv.T),
            "wpT": b16(Wproj[:, j * O:(j + 1) * O].T),  # [O, C]
            "cosq": b16(cosq), "sinq": b16(sinq),
        }
        if not (N == M):
            m["cosk"], m["sink"] = b16(cosk), b16(sink)
        in_maps.append(m)
    return in_maps


def host_gather(results, bproj, cfg):
    outs = []
    for b in range(4):
        p0 = results[2 * b]["outT"]
        p1 = results[2 * b + 1]["outT"]
        outs.append((np.asarray(p0) + np.asarray(p1)).T + bproj[None, :])
    return np.stack(outs).astype(np.float32)


def build_nc(cfg):
    N, M, C, O = cfg.N, cfg.M, cfg.C, cfg.O
    CC, OC4, HPC = cfg.CC, cfg.OC4, cfg.HPC
    NPB, NBL = cfg.NPB, cfg.NBL
    n_pb, m_pb = N // NPB, M // NPB
    MC = M // P
    NH = N // NBL
    GPI = MC // 2  # 2-mc groups per (nh, c4) iteration

    nc = bacc.Bacc("TRN2", target_bir_lowering=False, debug=False)
    xT = nc.dram_tensor("xT", [C, N], F16, kind="ExternalInput").ap()
    cT = nc.dram_tensor("cT", [C, M], F16, kind="ExternalInput").ap()
    wqT = nc.dram_tensor("wqT", [C, O], F16, kind="ExternalInput").ap()
    wkT = nc.dram_tensor("wkT", [C, O], F16, kind="ExternalInput").ap()
    wvT = nc.dram_tensor("wvT", [C, O], F16, kind="ExternalInput").ap()
    wpT = nc.dram_tensor("wpT", [O, C], F16, kind="ExternalInput").ap()
    cosq = nc.dram_tensor("cosq", [P, N], F16, kind="ExternalInput").ap()
    sinq = nc.dram_tensor("sinq", [P, N], F16, kind="ExternalInput").ap()
    if N == M:
        cosk, sink = cosq, sinq
    else:
        cosk = nc.dram_tensor("cosk", [P, M], F16, kind="ExternalInput").ap()
        sink = nc.dram_tensor("sink", [P, M], F16, kind="ExternalInput").ap()
    outT = nc.dram_tensor("outT", [C, N], F32, kind="ExternalOutput").ap()

    Exp = mybir.ActivationFunctionType.Exp
    Copy = mybir.ActivationFunctionType.Copy
    Mult = mybir.AluOpType.mult
    Add = mybir.AluOpType.add
    dma = nc.sync.dma_start
    A_dve = EXP_A * cfg.scale

    with tile.TileContext(nc) as tc:
        with tc.tile_pool(name="persist", bufs=1) as pp:
            # ---- persistent tiles (~80 KB/partition)
            qT = pp.tile([P, OC4, N], F16, tag="qT")
            kT = pp.tile([P, OC4, M], F16, tag="kT")
            # per head 128 stationary cols: col 0 = ones (den -> psO
            # partition 0), cols 64..127 = v (attn -> psO partitions
            # 64..127; engine APs need 32-aligned partition bases).
            vAll = pp.tile([P, MC, HPC * 128], BF16, tag="vAll")
            attnT = pp.tile([P, OC4, N], F16, tag="attnT")
            # ones column holds 1/128: psO[0] = den/128, and the psO
            # eviction scales by 1/128 so the un-normalized fp16 copy
            # cannot overflow (raw num reaches ~2e6 > fp16 max).
            nc.vector.memset(vAll[:, :, :], 1.0 / 128.0)

            # Attention-phase SBUF pools open FIRST so their regions are
            # disjoint from the phase-1 pools stacked above them: exp/PV
            # then carry no false deps on phase-1's last SBUF readers.
            with (
                tc.tile_pool(name="wpp", bufs=1) as wpp_pool,
                tc.tile_pool(name="exps", bufs=6) as exp_pool,
                tc.tile_pool(name="recipp", bufs=2) as recip_pool,
                tc.tile_pool(name="oev", bufs=3) as oev_pool,
            ):
                wp_sb = wpp_pool.tile([P, OC4, C], F16, tag="wp_sb")

                # ====== phase 1: K+V then Q projections, RoPE inline ======
                with (
                    tc.tile_pool(name="wqkv", bufs=1) as wqkv_pool,
                    tc.tile_pool(name="xc", bufs=2) as xc_pool,
                    tc.tile_pool(name="xq", bufs=2) as xq_pool,
                    tc.tile_pool(name="rope", bufs=2) as rope_pool,
                    tc.tile_pool(name="ps1", bufs=1, space="PSUM") as ps1,
                ):
                    wk_sb = wqkv_pool.tile([P, CC, O], F16, tag="wk_sb")
                    wv_sb = wqkv_pool.tile([P, CC, O], F16, tag="wv_sb")
                    wq_sb = wqkv_pool.tile([P, CC, O], F16, tag="wq_sb")
                    cos_q = wqkv_pool.tile([P, N], F16, tag="cos_q")
                    sin_q = wqkv_pool.tile([P, N], F16, tag="sin_q")
                    if N == M:
                        cos_k, sin_k = cos_q, sin_q
                    else:
                        cos_k = wqkv_pool.tile([P, M], F16, tag="cos_k")
                        sin_k = wqkv_pool.tile([P, M], F16, tag="sin_k")
                    wkr = wkT.rearrange("(cc p) o -> p cc o", p=P)
                    wvr = wvT.rearrange("(cc p) o -> p cc o", p=P)
                    wqr = wqT.rearrange("(cc p) o -> p cc o", p=P)

                    def rope_block(t, c4, sl, cos_t, sin_t):
                        sw = rope_pool.tile([P, NPB], F16, tag="rope_sw",
                                            name="sw")
                        nc.vector.stream_shuffle(
                            sw[:, :].bitcast(U32),
                            t[:, c4, sl].bitcast(U32), SHUFFLE_MASK)
                        t1 = rope_pool.tile([P, NPB], F16, tag="rope_t1",
                                            name="t1")
                        nc.vector.tensor_mul(t1[:, :], t[:, c4, sl],
                                             cos_t[:, sl])
                        nc.vector.tensor_mul(sw[:, :], sw[:, :],
                                             sin_t[:, sl])
                        nc.vector.tensor_add(t[:, c4, sl], t1[:, :],
                                             sw[:, :])

                    # ---- K+V projections fused (share context loads)
                    n_mc2 = NPB // P
                    for mb in range(m_pb):
                        ms = bass.ts(mb, NPB)
                        psk = [ps1.tile([P, NPB], F32, tag=f"psk{i}",
                                        name=f"psk{mb}_{i}")
                               for i in range(2)]
                        psv = [ps1.tile([P, O], F32, tag=f"psv{i}",
                                        name=f"psv{mb}_{i}")
                               for i in range(n_mc2)]
                        c_tiles = [xc_pool.tile([P, NPB], F16, tag=f"c{cc}",
                                                name=f"c{mb}_{cc}")
                                   for cc in range(CC)]
                        # mb=0: interleave weight-chunk DMAs with the
                        # context tiles so the first matmul starts early.
                        for cc in range(CC):
                            dma(c_tiles[cc][:, :], cT[bass.ts(cc, P), ms])
                            if mb == 0:
                                dma(wk_sb[:, cc, :], wkr[:, cc, :])
                                dma(wv_sb[:, cc, :], wvr[:, cc, :])
                        if mb == 0:
                            dma(cos_q[:, :], cosq)
                            dma(sin_q[:, :], sinq)
                            if N != M:
                                dma(cos_k[:, :], cosk)
                                dma(sin_k[:, :], sink)
                        for half in range(2):
                            for cc in range(CC):
                                for i in range(2):
                                    c4 = 2 * half + i
                                    nc.tensor.matmul(
                                        psk[i][:, :],
                                        wk_sb[:, cc, bass.ts(c4, P)],
                                        c_tiles[cc][:, :],
                                        start=(cc == 0), stop=(cc == CC - 1),
                                    )
                            for i in range(2):
                                c4 = 2 * half + i
                                nc.scalar.copy(kT[:, c4, ms], psk[i][:, :])
                                rope_block(kT, c4, ms, cos_k, sin_k)
                            if half == 0:
                                psk = [ps1.tile([P, NPB], F32, tag=f"psk{i}",
                                                name=f"pskb{mb}_{i}")
                                       for i in range(2)]
                        for cc in range(CC):
                            for mc2 in range(n_mc2):
                                nc.tensor.matmul(
                                    psv[mc2][:, :],
                                    c_tiles[cc][:, bass.ts(mc2, P)],
                                    wv_sb[:, cc, :],
                                    start=(cc == 0), stop=(cc == CC - 1),
                                )
                        for mc2 in range(n_mc2):
                            mc = mb * n_mc2 + mc2
                            nc.vector.tensor_copy(
                                vAll[:, mc, :].rearrange(
                                    "p (h e) -> p h e", e=128)[:, :, 64:128],
                                psv[mc2][:, :].rearrange(
                                    "p (h d) -> p h d", d=64),
                            )

                    for cc in range(CC):
                        dma(wq_sb[:, cc, :], wqr[:, cc, :])
                    dma(wp_sb[:, :, :],
                        wpT.rearrange("(oc p) e -> p oc e", p=P))

                    # ---- Q projection (2 banks, two c4-pair passes per nb)
                    for nb in range(n_pb):
                        ns = bass.ts(nb, NPB)
                        x_tiles = [xq_pool.tile([P, NPB], F16, tag=f"x{cc}",
                                                name=f"x{nb}_{cc}")
                                   for cc in range(CC)]
                        for cc in range(CC):
                            dma(x_tiles[cc][:, :], xT[bass.ts(cc, P), ns])
                        for half in range(2):
                            psq = [ps1.tile([P, NPB], F32, tag=f"psk{i}",
                                            name=f"psq{nb}_{half}_{i}")
                                   for i in range(2)]
                            for cc in range(CC):
                                for i in range(2):
                                    c4 = 2 * half + i
                                    nc.tensor.matmul(
                                        psq[i][:, :],
                                        wq_sb[:, cc, bass.ts(c4, P)],
                                        x_tiles[cc][:, :],
                                        start=(cc == 0), stop=(cc == CC - 1),
                                    )
                            for i in range(2):
                                c4 = 2 * half + i
                                nc.scalar.copy(qT[:, c4, ns], psq[i][:, :])
                                rope_block(qT, c4, ns, cos_q, sin_q)

                # ===== attention: flat pipeline over (nh, c4) + out-proj ==
                with (
                    # PSUM pool-open order fixes the bank map: psP 0-1 (on
                    # Q's banks), psO 2-3, psS 4-7 (free at KV end).
                    tc.tile_pool(name="psP", bufs=2, space="PSUM") as psP_pool,
                    tc.tile_pool(name="psO", bufs=1, space="PSUM") as psO_pool,
                    tc.tile_pool(name="psS", bufs=2, space="PSUM") as psS_pool,
                ):
                    iters = [(nh, c4) for nh in range(NH)
                             for c4 in range(OC4)]

                    def make_emitter(nh, c4):
                        nsl = bass.ts(nh, NBL)

                        def emit_group(g):
                            """S pairs for mc=2g,2g+1 (disjoint 64-row PE
                            groups, one 2-bank psS tile each) + one merged
                            [128,1024] exp per mc."""
                            out = []
                            for mc in (2 * g, 2 * g + 1):
                                msl = bass.ts(mc, P)
                                psS = psS_pool.tile(
                                    [P, 2, NBL], F32, tag="psS",
                                    name=f"psS_{nh}_{c4}_{mc}")
                                nc.tensor.matmul(
                                    psS[:, 0, :], kT[0:64, c4, msl],
                                    qT[0:64, c4, nsl],
                                    start=True, stop=True)
                                nc.tensor.matmul(
                                    psS[:, 1, :], kT[64:128, c4, msl],
                                    qT[64:128, c4, nsl],
                                    start=True, stop=True)
                                eS = exp_pool.tile(
                                    [P, 2, NBL], BF16, tag="eS",
                                    name=f"eS_{nh}_{c4}_{mc}")
                                if mc in DVE_MCS:
                                    nc.vector.tensor_scalar(
                                        eS[:, :, :].bitcast(I16),
                                        psS[:, :, :],
                                        A_dve, EXP_B, Mult, Add)
                                else:
                                    nc.scalar.activation(
                                        eS[:, :, :], psS[:, :, :], Exp,
                                        scale=cfg.scale)
                                out.append(eS)
                            return out

                        return emit_group

                    emitters = [make_emitter(nh, c4) for nh, c4 in iters]
                    esq = [emitters[0](0), emitters[0](1)]
                    for idx, (nh, c4) in enumerate(iters):
                        nsl = bass.ts(nh, NBL)
                        h1, h2 = 2 * c4, 2 * c4 + 1
                        psO1 = psO_pool.tile([P, NBL], F32, tag="psO1",
                                             name="psO1")
                        psO2 = psO_pool.tile([P, NBL], F32, tag="psO2",
                                             name="psO2")
                        for g in range(GPI):
                            # lookahead 2 groups; the last two slots carry
                            # the NEXT iteration's warmup groups so they
                            # interleave with this iteration's PV tail
                            # instead of arriving as a stalled burst.
                            if g + 2 < GPI:
                                esq.append(emitters[idx](g + 2))
                            elif idx + 1 < len(iters):
                                esq.append(emitters[idx + 1](g + 2 - GPI))
                            eA, eB = esq.pop(0)
                            for j, eS in ((0, eA), (1, eB)):
                                mc = 2 * g + j
                                st, sp = (mc == 0), (mc == MC - 1)
                                nc.tensor.matmul(
                                    psO1[:, :],
                                    vAll[:, mc, bass.ds(h1 * 128, 128)],
                                    eS[:, 0, :], start=st, stop=sp)
                                nc.tensor.matmul(
                                    psO2[:, :],
                                    vAll[:, mc, bass.ds(h2 * 128, 128)],
                                    eS[:, 1, :], start=st, stop=sp)

                        # normalize + evict (per head); den on psO part. 0.
                        # psO is released by the copy+recip pair (parallel
                        # engines, ~0.8us); the broadcast + in-place mul
                        # run later on SBUF, off the psO critical path, so
                        # the next iteration's first PV is not stalled.
                        for hh, psO in ((0, psO1), (1, psO2)):
                            rows = slice(hh * 64, hh * 64 + 64)
                            rc = recip_pool.tile([1, NBL], F32,
                                                 tag=f"rc{hh}", name="rc")
                            nc.vector.reciprocal_approx_fast(rc[:, :],
                                                             psO[0:1, :])
                            nc.scalar.activation(attnT[rows, c4, nsl],
                                                 psO[64:128, :], Copy,
                                                 scale=1.0 / 128.0)
                            rb = recip_pool.tile([P, NBL], F32,
                                                 tag=f"rb{hh}", name="rb")
                            nc.gpsimd.partition_broadcast(rb[:, :],
                                                          rc[:, :])
                            nc.vector.tensor_mul(attnT[rows, c4, nsl],
                                                 attnT[rows, c4, nsl],
                                                 rb[rows, :])

                        # ---- out-proj for this nh (overlaps nh+1)
                        if c4 == OC4 - 1:
                            for ec in range(C // P):
                                ps = psP_pool.tile([P, NBL], F32,
                                                   tag="ps_out")
                                for oc in range(OC4):
                                    nc.tensor.matmul(
                                        ps[:, :],
                                        wp_sb[:, oc, bass.ts(ec, P)],
                                        attnT[:, oc, nsl],
                                        start=(oc == 0),
                                        stop=(oc == OC4 - 1),
                                    )
                                ot = oev_pool.tile([P, NBL], F32, tag="ot")
                                if ec % 2 == 0:
                                    nc.scalar.copy(ot[:, :], ps[:, :])
                                else:
                                    nc.vector.tensor_copy(ot[:, :],
                                                          ps[:, :])
                                dma(outT[bass.ts(ec, P), nsl], ot[:, :])

    nc.compile()
    return nc


# ---------------------------------------------------------------- emulation
def _bf(a):
    return np.asarray(a).astype(BFNP).astype(np.float32)


def _f16(a):
    return np.asarray(a).astype(np.float16).astype(np.float32)


def _schraudolph(x):
    """Emulate the DVE bf16 exp trick (truncating f32->i16 cast)."""
    i = np.floor(EXP_A * x + EXP_B).astype(np.int16)
    return i.view(BFNP).astype(np.float32)


def emulate_core(m, cfg):
    """Numpy replica of the device program (layout + numerics validation)."""
    N, M, C, O = cfg.N, cfg.M, cfg.C, cfg.O
    xT = _f16(m["xT"])
    cT = _f16(m["cT"])
    qT = _f16(_f16(m["wqT"]).T @ xT)
    kT = _f16(_f16(m["wkT"]).T @ cT)
    v = _bf(cT.T @ _f16(m["wvT"]))
    cosk = _f16(m.get("cosk", m["cosq"]))
    sink = _f16(m.get("sink", m["sinq"]))
    cosq_t, sinq_t = _f16(m["cosq"]), _f16(m["sinq"])

    def rope(tT, cos, sin, L):
        t = tT.reshape(cfg.OC4, P, L)
        out = np.empty_like(t)
        for c4 in range(cfg.OC4):
            blk = t[c4]
            sw = np.empty_like(blk)
            for s in range(4):
                for i in range(32):
                    sw[s * 32 + i] = blk[s * 32 + SHUFFLE_MASK[i]]
            out[c4] = _f16(_f16(blk * cos) + _f16(sw * sin))
        return out.reshape(O, L)

    qT = rope(qT, cosq_t, sinq_t, N)
    kT = rope(kT, cosk, sink, M)

    attnT = np.empty((O, N), np.float32)
    for h in range(cfg.HPC):
        qh = qT[h * 64:(h + 1) * 64, :]
        kh = kT[h * 64:(h + 1) * 64, :]
        S = kh.T @ qh
        E = np.empty((M, N), np.float32)
        for mc in range(M // P):
            sl = slice(mc * P, (mc + 1) * P)
            if mc in DVE_MCS:
                E[sl] = _schraudolph(cfg.scale * S[sl])
            else:
                E[sl] = _bf(np.exp(cfg.scale * S[sl]))
        vh = v[:, h * 64:(h + 1) * 64]
        num = vh.T @ E
        den = E.sum(axis=0)
        # device: fp16 copy of num/128, then * (128/den)
        attnT[h * 64:(h + 1) * 64] = _f16(
            _f16(num / 128.0) * (128.0 / den)[None, :])
    return _f16(m["wpT"]).T.astype(np.float32) @ attnT


# ---------------------------------------------------------------- driver
_NC_CACHE = {}


def _get_nc(cfg):
    key = (cfg.N, cfg.M)
    if key not in _NC_CACHE:
        _NC_CACHE[key] = build_nc(cfg)
    return _NC_CACHE[key]


def _run(inputs, trace=False):
    cfg = CFG()
    nc = _get_nc(cfg)
    in_maps = host_prep(
        np.asarray(inputs["x"], np.float32),
        np.asarray(inputs["context"], np.float32),
        np.asarray(inputs["freqs_cis"], np.float32),
        np.asarray(inputs["Wq"], np.float32),
        np.asarray(inputs["Wkv"], np.float32),
        np.asarray(inputs["Wproj"], np.float32),
        cfg,
    )
    res = run_bass_kernel_spmd(nc, in_maps, list(range(8)), trace=trace)
    out = host_gather(res.results, np.asarray(inputs["bproj"], np.float32), cfg)
    return out, res


def kernel(**inputs):
    out, _ = _run(inputs, trace=False)
    return out


def timed_run(inputs):
    _, res = _run(inputs, trace=True)
    return res.exec_time_ns, res
